# revision 19
# baseline (speedup 1.0000x reference)
"""Per-neuron grouped MLP (conv-style) kernel for Trainium2, 8 NeuronCores.

Math (per group d):  h = x[:, d, :] @ W1[d].T + b1[d]; g = gelu(h); out[:, d] = g @ W2[d] + b2[d]
  x: [B=512, D=2048, M=128], W1: [D, H=128, M], b1: [D, H], W2: [D, H], b2: [D]

Strategy (v2):
  - Shard on D: each of 8 cores owns D_LOC = 256 independent per-neuron MLPs.
  - x is quantized to int8 on host with a per-(d,m) scale folded into W1
    (W1'[m,d,h] = W1[d,h,m] * sx[d,m]); the DMA casts int8 -> fp16 in the
    SDMA datapath, so HBM traffic for x is 1 byte/elem while the matmul
    runs in fp16 on integer-valued activations.
  - DRAM layouts are [M, D_LOC, *] so every per-supergroup DMA reads a
    contiguous per-partition chunk (8KB x / 4KB w1).
  - Per pair of 2 d's: 2 matmuls into one [H, 2B] fp32 psum tile, one Gelu
    activation (ScalarE, exact erf) psum -> fp16 g in SBUF.
  - Per quad of 4 d's: 4 packed MM2s, tile_position=(0,32j) -> psum rows
    {0,32,64,96}; DVE copies psum -> fp16 o_sb; strided-partition DMA to
    outT fp16. b2 and the fp32 upcast happen on host.
"""

import numpy as np

B, D, M, H = 512, 2048, 128, 128
N_CORES = 8
D_LOC = D // N_CORES  # 256
QUAD = 4     # d's per MM2 packing group
PAIR = 2     # d's per psum1/ACT batch
SUPER = 16   # d's per super-group: one x DMA, one w1 DMA, one out DMA
# Within a super-group [D0, D0+16), quad c (c=0..3) handles d = D0 + 4j + c
# (j=0..3); MM2 j lands on psum row 32j, so out rows {D0..D0+15} are exactly
# o_sb[0::32, c, :] in (row, quad, b) iteration order -> single strided DMA.

X_INT8 = True  # False: ship x as fp16 (no quantization)
ACT_QUAD = False  # True: one gelu instruction per quad (4-bank psum tiles)

_NC_CACHE = {}


def build_nc(bias_mode: bool, x_int8: bool = X_INT8, reps: int = 1,
             act_quad: bool | None = None):
    if act_quad is None:
        act_quad = ACT_QUAD
    key = (bias_mode, x_int8, reps, act_quad)
    if key in _NC_CACHE:
        return _NC_CACHE[key]

    import concourse.bacc as bacc
    import concourse.mybir as mybir
    import concourse.tile as tile

    f32 = mybir.dt.float32
    f16 = mybir.dt.float16
    xdt = mybir.dt.int8 if x_int8 else f16
    GELU = mybir.ActivationFunctionType.Gelu

    nc = bacc.Bacc("TRN2", target_bir_lowering=False, debug=False, num_devices=N_CORES)
    xT = nc.dram_tensor("xT", [M, D_LOC, B], xdt, kind="ExternalInput").ap()
    w1T = nc.dram_tensor("w1T", [M, D_LOC, H], f16, kind="ExternalInput").ap()
    w2T = nc.dram_tensor("w2T", [H, D_LOC], f16, kind="ExternalInput").ap()
    b1T = nc.dram_tensor("b1T", [H, D_LOC], f32, kind="ExternalInput").ap()
    outT = nc.dram_tensor("outT", [D_LOC, B], f16, kind="ExternalOutput").ap()

    ps1_shape = [H, (QUAD if act_quad else PAIR) * B]
    ps1_bufs = 2 if act_quad else 3
    with (
        tile.TileContext(nc) as tc,
        tc.tile_pool(name="singles", bufs=1) as singles,
        tc.tile_pool(name="xp", bufs=3) as xp,
        tc.tile_pool(name="wp", bufs=2) as wp,
        tc.tile_pool(name="gp", bufs=4) as gp,
        tc.tile_pool(name="op", bufs=4) as op_pool,
        tc.tile_pool(name="ps1", bufs=ps1_bufs, space="PSUM") as ps1,
        tc.tile_pool(name="ps2", bufs=2, space="PSUM") as ps2,
    ):
        w2_sb = singles.tile([H, D_LOC], f16)
        nc.sync.dma_start(out=w2_sb[:], in_=w2T[:])
        b1_sb = None
        if bias_mode:
            b1_sb = singles.tile([H, D_LOC], f32)
            nc.sync.dma_start(out=b1_sb[:], in_=b1T[:])

        for _rep in range(reps):
            _body_loop(nc, tc, bias_mode, f16, f32, GELU,
                       xT, w1T, outT, w2_sb, b1_sb,
                       xp, wp, gp, op_pool, ps1, ps2, act_quad)

    nc.compile()
    _NC_CACHE[key] = nc
    return nc


def _sg_plan():
    """Supergroup sizes: small at the start (fast pipeline fill: compute can
    begin after a 262KB DMA instead of 2.1MB) and at the end (short tail)."""
    sizes = [4, 4, 8] + [SUPER] * ((D_LOC - 32) // SUPER) + [8, 4, 4]
    assert sum(sizes) == D_LOC
    out, d0 = [], 0
    for s in sizes:
        out.append((d0, s))
        d0 += s
    return out


def _body_loop(nc, tc, bias_mode, f16, f32, GELU, xT, w1T, outT, w2_sb, b1_sb,
               xp, wp, gp, op_pool, ps1, ps2, act_quad=False):
    NPAIR_Q = QUAD // PAIR  # 2 pairs per quad
    sgs = _sg_plan()
    # pair index -> (sg, c, pr); within sg of NQ quads, quad c handles
    # d = D0 + NQ*(PAIR*pr + j) + c
    pairs = [
        (sgi, c, pr)
        for sgi, (D0, size) in enumerate(sgs)
        for c in range(size // QUAD)
        for pr in range(NPAIR_Q)
    ]

    sg_state = {}   # sgi -> (x_sb, w1_sb, o_sb)
    quad_g = {}     # (sgi, c) -> g_sb
    quad_p = {}     # (sgi, c) -> quad psum tile (act_quad mode)

    def emit_mm1(pi):
        """Stage 1: (DMA loads at supergroup start) + 2 MM1 matmuls."""
        sgi, c, pr = pairs[pi]
        D0, size = sgs[sgi]
        NQ = size // QUAD
        if c == 0 and pr == 0:
            x_sb = xp.tile([M, size, B], f16, name=f"x_{size}")
            nc.gpsimd.dma_start(out=x_sb[:], in_=xT[:, D0 : D0 + size, :])
            w1_sb = wp.tile([M, size, H], f16, name=f"w1_{size}")
            nc.sync.dma_start(out=w1_sb[:], in_=w1T[:, D0 : D0 + size, :])
            o_sb = op_pool.tile([128, NQ, B], f16, name=f"o_{size}")
            sg_state[sgi] = (x_sb, w1_sb, o_sb)
        x_sb, w1_sb, _ = sg_state[sgi]
        if act_quad:
            if pr == 0:
                pq = ps1.tile([H, QUAD * B], f32, name="p_quad")
                quad_p[(sgi, c)] = pq
            p1 = quad_p[(sgi, c)][:, pr * PAIR * B : (pr + 1) * PAIR * B]
        else:
            p1 = ps1.tile([H, PAIR * B], f32)
        for j in range(PAIR):
            jj = PAIR * pr + j
            nc.tensor.matmul(
                p1[:, j * B : (j + 1) * B],
                lhsT=w1_sb[:, NQ * jj + c, :],
                rhs=x_sb[:, NQ * jj + c, :],
                start=True,
                stop=True,
            )
        return p1

    def emit_consume(pi, p1):
        """Stage 2: gelu; at quad end also MM2 -> DVE copy -> out DMA."""
        sgi, c, pr = pairs[pi]
        D0, size = sgs[sgi]
        NQ = size // QUAD
        _, _, o_sb = sg_state[sgi]
        if pr == 0:
            g_new = gp.tile([H, QUAD * B], f16, name="g_quad")
            quad_g[(sgi, c)] = g_new
        g_sb = quad_g[(sgi, c)]
        if act_quad and pr != NPAIR_Q - 1:
            return  # whole-quad gelu happens on the last pair
        if bias_mode:
            prs = range(NPAIR_Q) if act_quad else [pr]
            pq = quad_p[(sgi, c)] if act_quad else None
            for pr2 in prs:
                psl = (pq[:, pr2 * PAIR * B : (pr2 + 1) * PAIR * B]
                       if act_quad else p1)
                for j in range(PAIR):
                    dd = D0 + NQ * (PAIR * pr2 + j) + c
                    nc.scalar.activation(
                        g_sb[:, (PAIR * pr2 + j) * B : (PAIR * pr2 + j + 1) * B],
                        psl[:, j * B : (j + 1) * B],
                        GELU,
                        bias=b1_sb[:, dd : dd + 1],
                    )
        elif act_quad:
            pq = quad_p[(sgi, c)]
            nc.scalar.activation(g_sb[:], pq[:], GELU)
        else:
            gsl = g_sb[:, pr * PAIR * B : (pr + 1) * PAIR * B]
            nc.scalar.activation(gsl[:], p1[:], GELU)
        if pr != NPAIR_Q - 1:
            return
        # quad complete: MM2 (4 col-tiled 1-row matmuls) + DVE copy
        del quad_g[(sgi, c)]
        if act_quad:
            # reuse the (consumed) last bank of the quad psum tile
            pq = quad_p.pop((sgi, c))
            p2 = pq[:, (QUAD - 1) * B : QUAD * B]
        else:
            p2 = ps2.tile([128, B], f32)
        for j in range(QUAD):
            dd = D0 + NQ * j + c
            nc.tensor.matmul(
                p2[32 * j : 32 * j + 1, :],
                lhsT=w2_sb[:, dd : dd + 1],
                rhs=g_sb[:, j * B : (j + 1) * B],
                start=True,
                stop=True,
                tile_position=(0, 32 * j),
            )
        nc.vector.tensor_copy(o_sb[:, c, :], p2[:])
        if c == NQ - 1:
            nc.sync.dma_start(
                out=outT[D0 : D0 + size, :], in_=o_sb[0::32, :, :]
            )
            del sg_state[sgi]

    # 1-deep software pipeline at pair granularity: PE runs MM1(p+1)
    # while ACT consumes pair p.
    prev = emit_mm1(0)
    for pi in range(len(pairs)):
        if pi + 1 < len(pairs):
            nxt = emit_mm1(pi + 1)
        emit_consume(pi, prev)
        prev = nxt if pi + 1 < len(pairs) else None


def prepare_in_maps(x, W1, b1, W2, x_int8: bool = X_INT8):
    """Host-side shard + transpose (+ int8 quantization). 8 per-core dicts."""
    x = np.asarray(x, dtype=np.float32)
    W1 = np.asarray(W1, dtype=np.float32)
    b1 = np.asarray(b1, dtype=np.float32)
    W2 = np.asarray(W2, dtype=np.float32)

    in_maps = []
    for k in range(N_CORES):
        sl = slice(k * D_LOC, (k + 1) * D_LOC)
        xk = x[:, sl, :]  # [B, D_LOC, M]
        w1k = W1[sl]      # [D_LOC, H, M]
        if x_int8:
            sx = np.abs(xk).max(axis=0) / 127.0          # [D_LOC, M]
            sx = np.maximum(sx, 1e-12)
            xq = np.rint(xk / sx[None]).astype(np.int8)  # [B, D_LOC, M]
            xT_k = np.ascontiguousarray(xq.transpose(2, 1, 0))          # [M, D_LOC, B]
            w1s = w1k * sx[:, None, :]                   # [D_LOC, H, M] * sx[d,m]
        else:
            xT_k = np.ascontiguousarray(
                xk.transpose(2, 1, 0), dtype=np.float16
            )
            w1s = w1k
        w1T_k = np.ascontiguousarray(w1s.transpose(2, 0, 1), dtype=np.float16)  # [M, D_LOC, H]
        w2T_k = np.ascontiguousarray(W2[sl].T, dtype=np.float16)
        b1T_k = np.ascontiguousarray(b1[sl].T, dtype=np.float32)
        in_maps.append({"xT": xT_k, "w1T": w1T_k, "w2T": w2T_k, "b1T": b1T_k})
    return in_maps


def assemble_output(results, b2):
    outT_full = np.concatenate([r["outT"] for r in results], axis=0)  # [D, B] f16
    out = outT_full.T.astype(np.float32)  # [B, D]
    b2 = np.asarray(b2, dtype=np.float32)
    if np.any(b2):
        out = out + b2[None, :]
    return np.ascontiguousarray(out)


def kernel(pre_activation_history, W1, b1, W2, b2):
    from concourse.bass_utils import run_bass_kernel_spmd

    b1 = np.asarray(b1, dtype=np.float32)
    bias_mode = bool(np.any(b1))
    nc = build_nc(bias_mode)
    in_maps = prepare_in_maps(pre_activation_history, W1, b1, W2)
    res = run_bass_kernel_spmd(nc, in_maps, core_ids=list(range(N_CORES)))
    return assemble_output(res.results, b2)


# revision 20
# speedup vs baseline: 1.3138x; 1.3138x over previous
"""Per-neuron grouped MLP (conv-style) kernel for Trainium2, 8 NeuronCores.

Math (per group d):  h = x[:, d, :] @ W1[d].T + b1[d]; g = gelu(h); out[:, d] = g @ W2[d] + b2[d]
  x: [B=512, D=2048, M=128], W1: [D, H=128, M], b1: [D, H], W2: [D, H], b2: [D]

Strategy (v2):
  - Shard on D: each of 8 cores owns D_LOC = 256 independent per-neuron MLPs.
  - x is quantized to int8 on host with a per-(d,m) scale folded into W1
    (W1'[m,d,h] = W1[d,h,m] * sx[d,m]); the DMA casts int8 -> fp16 in the
    SDMA datapath, so HBM traffic for x is 1 byte/elem while the matmul
    runs in fp16 on integer-valued activations.
  - DRAM layouts are [M, D_LOC, *] so every per-supergroup DMA reads a
    contiguous per-partition chunk (8KB x / 4KB w1).
  - Per pair of 2 d's: 2 matmuls into one [H, 2B] fp32 psum tile, one Gelu
    activation (ScalarE, exact erf) psum -> fp16 g in SBUF.
  - Per quad of 4 d's: 4 packed MM2s, tile_position=(0,32j) -> psum rows
    {0,32,64,96}; DVE copies psum -> fp16 o_sb; strided-partition DMA to
    outT fp16. b2 and the fp32 upcast happen on host.
"""

import numpy as np

B, D, M, H = 512, 2048, 128, 128
N_CORES = 8
D_LOC = D // N_CORES  # 256
QUAD = 4     # d's per MM2 packing group
PAIR = 2     # d's per psum1/ACT batch
SUPER = 16   # d's per super-group: one x DMA, one w1 DMA, one out DMA
# Within a super-group [D0, D0+16), quad c (c=0..3) handles d = D0 + 4j + c
# (j=0..3); MM2 j lands on psum row 32j, so out rows {D0..D0+15} are exactly
# o_sb[0::32, c, :] in (row, quad, b) iteration order -> single strided DMA.

X_INT8 = True  # False: ship x as fp16 (no quantization)
ACT_QUAD = False  # True: one gelu instruction per quad (4-bank psum tiles)

_NC_CACHE = {}


def build_nc(bias_mode: bool, x_int8: bool = X_INT8, reps: int = 1,
             act_quad: bool | None = None):
    if act_quad is None:
        act_quad = ACT_QUAD
    key = (bias_mode, x_int8, reps, act_quad)
    if key in _NC_CACHE:
        return _NC_CACHE[key]

    import concourse.bacc as bacc
    import concourse.mybir as mybir
    import concourse.tile as tile

    f32 = mybir.dt.float32
    f16 = mybir.dt.float16
    xdt = mybir.dt.int8 if x_int8 else f16
    GELU = mybir.ActivationFunctionType.Gelu

    nc = bacc.Bacc("TRN2", target_bir_lowering=False, debug=False, num_devices=N_CORES)
    xT = nc.dram_tensor("xT", [M, D_LOC, B], xdt, kind="ExternalInput").ap()
    w1T = nc.dram_tensor("w1T", [M, D_LOC, H], f16, kind="ExternalInput").ap()
    w2T = nc.dram_tensor("w2T", [H, D_LOC], f16, kind="ExternalInput").ap()
    b1T = nc.dram_tensor("b1T", [H, D_LOC], f32, kind="ExternalInput").ap()
    outT = nc.dram_tensor("outT", [D_LOC, B], f16, kind="ExternalOutput").ap()

    ps1_shape = [H, (QUAD if act_quad else PAIR) * B]
    ps1_bufs = 2 if act_quad else 3
    with (
        tile.TileContext(nc) as tc,
        tc.tile_pool(name="singles", bufs=1) as singles,
        tc.tile_pool(name="xp", bufs=3) as xp,
        tc.tile_pool(name="wp", bufs=2) as wp,
        tc.tile_pool(name="gp", bufs=4) as gp,
        tc.tile_pool(name="op", bufs=4) as op_pool,
        tc.tile_pool(name="ps1", bufs=ps1_bufs, space="PSUM") as ps1,
        tc.tile_pool(name="ps2", bufs=2, space="PSUM") as ps2,
    ):
        w2_sb = singles.tile([H, D_LOC], f16)
        nc.sync.dma_start(out=w2_sb[:], in_=w2T[:])
        b1_sb = None
        if bias_mode:
            b1_sb = singles.tile([H, D_LOC], f32)
            nc.sync.dma_start(out=b1_sb[:], in_=b1T[:])
        # Dummy 1-col gelu with no input deps: hoists the ~2.7us
        # ACT_TABLE_LOAD for the gelu set under the initial DMAs.
        warm = singles.tile([128, 1], f32)
        nc.gpsimd.memset(warm[:], 0.0)
        nc.scalar.activation(warm[:], warm[:], GELU)

        for _rep in range(reps):
            _body_loop(nc, tc, bias_mode, f16, f32, GELU,
                       xT, w1T, outT, w2_sb, b1_sb,
                       xp, wp, gp, op_pool, ps1, ps2, act_quad)

    nc.compile()
    _NC_CACHE[key] = nc
    return nc


def _sg_plan():
    """Supergroup sizes: small at the start (fast pipeline fill: compute can
    begin after a 262KB DMA instead of 2.1MB) and at the end (short tail)."""
    sizes = [4, 4, 8] + [SUPER] * ((D_LOC - 32) // SUPER) + [8, 4, 4]
    assert sum(sizes) == D_LOC
    out, d0 = [], 0
    for s in sizes:
        out.append((d0, s))
        d0 += s
    return out


def _body_loop(nc, tc, bias_mode, f16, f32, GELU, xT, w1T, outT, w2_sb, b1_sb,
               xp, wp, gp, op_pool, ps1, ps2, act_quad=False):
    NPAIR_Q = QUAD // PAIR  # 2 pairs per quad
    sgs = _sg_plan()
    # pair index -> (sg, c, pr); within sg of NQ quads, quad c handles
    # d = D0 + NQ*(PAIR*pr + j) + c
    pairs = [
        (sgi, c, pr)
        for sgi, (D0, size) in enumerate(sgs)
        for c in range(size // QUAD)
        for pr in range(NPAIR_Q)
    ]

    sg_state = {}   # sgi -> (x_sb, w1_sb, o_sb)
    quad_g = {}     # (sgi, c) -> g_sb
    quad_p = {}     # (sgi, c) -> quad psum tile (act_quad mode)

    def emit_mm1(pi):
        """Stage 1: (DMA loads at supergroup start) + 2 MM1 matmuls."""
        sgi, c, pr = pairs[pi]
        D0, size = sgs[sgi]
        NQ = size // QUAD
        if c == 0 and pr == 0:
            x_sb = xp.tile([M, size, B], f16, name=f"x_{size}")
            nc.gpsimd.dma_start(out=x_sb[:], in_=xT[:, D0 : D0 + size, :])
            w1_sb = wp.tile([M, size, H], f16, name=f"w1_{size}")
            nc.sync.dma_start(out=w1_sb[:], in_=w1T[:, D0 : D0 + size, :])
            o_sb = op_pool.tile([128, NQ, B], f16, name=f"o_{size}")
            sg_state[sgi] = (x_sb, w1_sb, o_sb)
        x_sb, w1_sb, _ = sg_state[sgi]
        if act_quad:
            if pr == 0:
                pq = ps1.tile([H, QUAD * B], f32, name="p_quad")
                quad_p[(sgi, c)] = pq
            p1 = quad_p[(sgi, c)][:, pr * PAIR * B : (pr + 1) * PAIR * B]
        else:
            p1 = ps1.tile([H, PAIR * B], f32)
        for j in range(PAIR):
            jj = PAIR * pr + j
            nc.tensor.matmul(
                p1[:, j * B : (j + 1) * B],
                lhsT=w1_sb[:, NQ * jj + c, :],
                rhs=x_sb[:, NQ * jj + c, :],
                start=True,
                stop=True,
            )
        return p1

    def emit_consume(pi, p1):
        """Stage 2: gelu; at quad end also MM2 -> DVE copy -> out DMA."""
        sgi, c, pr = pairs[pi]
        D0, size = sgs[sgi]
        NQ = size // QUAD
        _, _, o_sb = sg_state[sgi]
        if pr == 0:
            g_new = gp.tile([H, QUAD * B], f16, name="g_quad")
            quad_g[(sgi, c)] = g_new
        g_sb = quad_g[(sgi, c)]
        if act_quad and pr != NPAIR_Q - 1:
            return  # whole-quad gelu happens on the last pair
        if bias_mode:
            prs = range(NPAIR_Q) if act_quad else [pr]
            pq = quad_p[(sgi, c)] if act_quad else None
            for pr2 in prs:
                psl = (pq[:, pr2 * PAIR * B : (pr2 + 1) * PAIR * B]
                       if act_quad else p1)
                for j in range(PAIR):
                    dd = D0 + NQ * (PAIR * pr2 + j) + c
                    nc.scalar.activation(
                        g_sb[:, (PAIR * pr2 + j) * B : (PAIR * pr2 + j + 1) * B],
                        psl[:, j * B : (j + 1) * B],
                        GELU,
                        bias=b1_sb[:, dd : dd + 1],
                    )
        elif act_quad:
            pq = quad_p[(sgi, c)]
            nc.scalar.activation(g_sb[:], pq[:], GELU)
        else:
            gsl = g_sb[:, pr * PAIR * B : (pr + 1) * PAIR * B]
            nc.scalar.activation(gsl[:], p1[:], GELU)
        if pr != NPAIR_Q - 1:
            return
        # quad complete: MM2 (4 col-tiled 1-row matmuls) + DVE copy
        del quad_g[(sgi, c)]
        if act_quad:
            # reuse the (consumed) last bank of the quad psum tile
            pq = quad_p.pop((sgi, c))
            p2 = pq[:, (QUAD - 1) * B : QUAD * B]
        else:
            p2 = ps2.tile([128, B], f32)
        for j in range(QUAD):
            dd = D0 + NQ * j + c
            nc.tensor.matmul(
                p2[32 * j : 32 * j + 1, :],
                lhsT=w2_sb[:, dd : dd + 1],
                rhs=g_sb[:, j * B : (j + 1) * B],
                start=True,
                stop=True,
                tile_position=(0, 32 * j),
            )
        nc.vector.tensor_copy(o_sb[:, c, :], p2[:])
        if c == NQ - 1:
            nc.sync.dma_start(
                out=outT[D0 : D0 + size, :], in_=o_sb[0::32, :, :]
            )
            del sg_state[sgi]

    # 1-deep software pipeline at pair granularity: PE runs MM1(p+1)
    # while ACT consumes pair p.
    prev = emit_mm1(0)
    for pi in range(len(pairs)):
        if pi + 1 < len(pairs):
            nxt = emit_mm1(pi + 1)
        emit_consume(pi, prev)
        prev = nxt if pi + 1 < len(pairs) else None


def prepare_in_maps(x, W1, b1, W2, x_int8: bool = X_INT8):
    """Host-side shard + transpose (+ int8 quantization). 8 per-core dicts."""
    x = np.asarray(x, dtype=np.float32)
    W1 = np.asarray(W1, dtype=np.float32)
    b1 = np.asarray(b1, dtype=np.float32)
    W2 = np.asarray(W2, dtype=np.float32)

    in_maps = []
    for k in range(N_CORES):
        sl = slice(k * D_LOC, (k + 1) * D_LOC)
        xk = x[:, sl, :]  # [B, D_LOC, M]
        w1k = W1[sl]      # [D_LOC, H, M]
        if x_int8:
            sx = np.abs(xk).max(axis=0) / 127.0          # [D_LOC, M]
            sx = np.maximum(sx, 1e-12)
            xq = np.rint(xk / sx[None]).astype(np.int8)  # [B, D_LOC, M]
            xT_k = np.ascontiguousarray(xq.transpose(2, 1, 0))          # [M, D_LOC, B]
            w1s = w1k * sx[:, None, :]                   # [D_LOC, H, M] * sx[d,m]
        else:
            xT_k = np.ascontiguousarray(
                xk.transpose(2, 1, 0), dtype=np.float16
            )
            w1s = w1k
        w1T_k = np.ascontiguousarray(w1s.transpose(2, 0, 1), dtype=np.float16)  # [M, D_LOC, H]
        w2T_k = np.ascontiguousarray(W2[sl].T, dtype=np.float16)
        b1T_k = np.ascontiguousarray(b1[sl].T, dtype=np.float32)
        in_maps.append({"xT": xT_k, "w1T": w1T_k, "w2T": w2T_k, "b1T": b1T_k})
    return in_maps


def assemble_output(results, b2):
    outT_full = np.concatenate([r["outT"] for r in results], axis=0)  # [D, B] f16
    out = outT_full.T.astype(np.float32)  # [B, D]
    b2 = np.asarray(b2, dtype=np.float32)
    if np.any(b2):
        out = out + b2[None, :]
    return np.ascontiguousarray(out)


def kernel(pre_activation_history, W1, b1, W2, b2):
    from concourse.bass_utils import run_bass_kernel_spmd

    b1 = np.asarray(b1, dtype=np.float32)
    bias_mode = bool(np.any(b1))
    nc = build_nc(bias_mode)
    in_maps = prepare_in_maps(pre_activation_history, W1, b1, W2)
    res = run_bass_kernel_spmd(nc, in_maps, core_ids=list(range(N_CORES)))
    return assemble_output(res.results, b2)


# revision 22
# speedup vs baseline: 1.3584x; 1.0339x over previous
"""Per-neuron grouped MLP (conv-style) kernel for Trainium2, 8 NeuronCores.

Math (per group d):  h = x[:, d, :] @ W1[d].T + b1[d]; g = gelu(h); out[:, d] = g @ W2[d] + b2[d]
  x: [B=512, D=2048, M=128], W1: [D, H=128, M], b1: [D, H], W2: [D, H], b2: [D]

Strategy (v4) — the kernel is ScalarE-bound: gelu on ACT runs at
1 elem/lane/cycle @1.2GHz regardless of dtype (HW-measured 1000ns per
[128,1024] activation), so B*D_LOC*H/128 = 131072 elems/partition set a
~128us/core floor; every other engine is scheduled to hide under it.

  - Shard on D: each of 8 cores owns D_LOC = 256 independent per-neuron MLPs.
  - x is quantized to int8 on host with a per-(d,m) scale folded into W1
    (W1'[m,d,h] = W1[d,h,m] * sx[d,m]); nc.gpsimd.dma_start casts
    int8 -> fp16 in the SDMA datapath, so HBM traffic for x is
    1 byte/elem while the matmul runs in fp16 on integer-valued
    activations (rel err ~7e-3, gate 2e-2).
  - DRAM layouts are [M, D_LOC, *] so every per-supergroup DMA reads a
    contiguous per-partition chunk (8KB x / 4KB w1).
  - Per pair of 2 d's: 2 matmuls into one [H, 2B] fp32 psum tile
    (ps1, bufs=3), one Gelu (exact erf) psum -> fp16 g in SBUF.
  - Per quad of 4 d's: 4 packed MM2s, tile_position=(0,32j) run
    concurrently in PE column groups -> psum rows {0,32,64,96} (ps2);
    DVE copies psum -> fp16 o_sb; strided-partition DMA to outT fp16.
    b2 and the fp32 upcast happen on host.
  - Software pipelining at pair granularity (MM1 of pair p+1 emitted
    before the consumers of pair p) keeps ACT streaming gaplessly;
    supergroups are sized [4,4,8,16...16,8,4,4] to shorten pipeline
    fill/drain, and a dummy 1-col gelu hoists the ~2.7us
    ACT_TABLE_LOAD under the initial DMAs.
  - Variants measured and rejected: quad-level ACT (fewer instruction
    overheads but Tile's whole-tile dependency tracking serializes
    MM1(k+2) behind the DVE read of the shared 4-bank psum tile:
    165us vs 125us); fp8 anywhere (3.6% rms error fails the gate);
    gelu on DVE/GPSIMD (no LUT; polynomial needs ~10 passes).
"""

import numpy as np

B, D, M, H = 512, 2048, 128, 128
N_CORES = 8
D_LOC = D // N_CORES  # 256
QUAD = 4     # d's per MM2 packing group
PAIR = 2     # d's per psum1/ACT batch
SUPER = 16   # d's per super-group: one x DMA, one w1 DMA, one out DMA
# Within a super-group [D0, D0+16), quad c (c=0..3) handles d = D0 + 4j + c
# (j=0..3); MM2 j lands on psum row 32j, so out rows {D0..D0+15} are exactly
# o_sb[0::32, c, :] in (row, quad, b) iteration order -> single strided DMA.

X_INT8 = True  # False: ship x as fp16 (no quantization)
ACT_QUAD = False  # True: one gelu instruction per quad (4-bank psum tiles)

_NC_CACHE = {}


def build_nc(bias_mode: bool, x_int8: bool = X_INT8, reps: int = 1,
             act_quad: bool | None = None):
    if act_quad is None:
        act_quad = ACT_QUAD
    key = (bias_mode, x_int8, reps, act_quad)
    if key in _NC_CACHE:
        return _NC_CACHE[key]

    import concourse.bacc as bacc
    import concourse.mybir as mybir
    import concourse.tile as tile

    f32 = mybir.dt.float32
    f16 = mybir.dt.float16
    xdt = mybir.dt.int8 if x_int8 else f16
    GELU = mybir.ActivationFunctionType.Gelu

    nc = bacc.Bacc("TRN2", target_bir_lowering=False, debug=False, num_devices=N_CORES)
    xT = nc.dram_tensor("xT", [M, D_LOC, B], xdt, kind="ExternalInput").ap()
    w1T = nc.dram_tensor("w1T", [M, D_LOC, H], f16, kind="ExternalInput").ap()
    w2T = nc.dram_tensor("w2T", [H, D_LOC], f16, kind="ExternalInput").ap()
    b1T = nc.dram_tensor("b1T", [H, D_LOC], f32, kind="ExternalInput").ap()
    outT = nc.dram_tensor("outT", [D_LOC, B], f16, kind="ExternalOutput").ap()

    ps1_bufs = 2 if act_quad else 3
    with (
        tile.TileContext(nc) as tc,
        tc.tile_pool(name="singles", bufs=1) as singles,
        tc.tile_pool(name="xp", bufs=3) as xp,
        tc.tile_pool(name="wp", bufs=2) as wp,
        tc.tile_pool(name="gp", bufs=4) as gp,
        tc.tile_pool(name="op", bufs=4) as op_pool,
        tc.tile_pool(name="ps1", bufs=ps1_bufs, space="PSUM") as ps1,
        tc.tile_pool(name="ps2", bufs=2, space="PSUM") as ps2,
    ):
        w2_sb = singles.tile([H, D_LOC], f16)
        nc.sync.dma_start(out=w2_sb[:], in_=w2T[:])
        b1_sb = None
        if bias_mode:
            b1_sb = singles.tile([H, D_LOC], f32)
            nc.sync.dma_start(out=b1_sb[:], in_=b1T[:])
        # Dummy 1-col gelu with no input deps: hoists the ~2.7us
        # ACT_TABLE_LOAD for the gelu set under the initial DMAs.
        warm = singles.tile([128, 1], f32)
        nc.gpsimd.memset(warm[:], 0.0)
        nc.scalar.activation(warm[:], warm[:], GELU)

        for _rep in range(reps):
            _body_loop(nc, tc, bias_mode, f16, f32, GELU,
                       xT, w1T, outT, w2_sb, b1_sb,
                       xp, wp, gp, op_pool, ps1, ps2, act_quad)

    nc.compile()
    _NC_CACHE[key] = nc
    return nc


def _sg_plan():
    """Supergroup sizes: small at the start (fast pipeline fill: compute can
    begin after a 262KB DMA instead of 2.1MB) and at the end (short tail)."""
    sizes = [4, 4, 8] + [SUPER] * ((D_LOC - 32) // SUPER) + [8, 4, 4]
    assert sum(sizes) == D_LOC
    out, d0 = [], 0
    for s in sizes:
        out.append((d0, s))
        d0 += s
    return out


def _body_loop(nc, tc, bias_mode, f16, f32, GELU, xT, w1T, outT, w2_sb, b1_sb,
               xp, wp, gp, op_pool, ps1, ps2, act_quad=False):
    NPAIR_Q = QUAD // PAIR  # 2 pairs per quad
    sgs = _sg_plan()
    # pair index -> (sg, c, pr); within sg of NQ quads, quad c handles
    # d = D0 + NQ*(PAIR*pr + j) + c
    pairs = [
        (sgi, c, pr)
        for sgi, (D0, size) in enumerate(sgs)
        for c in range(size // QUAD)
        for pr in range(NPAIR_Q)
    ]

    sg_state = {}   # sgi -> (x_sb, w1_sb, o_sb)
    quad_g = {}     # (sgi, c) -> g_sb
    quad_p = {}     # (sgi, c) -> quad psum tile (act_quad mode)

    def emit_mm1(pi):
        """Stage 1: (DMA loads at supergroup start) + 2 MM1 matmuls."""
        sgi, c, pr = pairs[pi]
        D0, size = sgs[sgi]
        NQ = size // QUAD
        if c == 0 and pr == 0:
            x_sb = xp.tile([M, size, B], f16, name=f"x_{size}")
            nc.gpsimd.dma_start(out=x_sb[:], in_=xT[:, D0 : D0 + size, :])
            w1_sb = wp.tile([M, size, H], f16, name=f"w1_{size}")
            nc.sync.dma_start(out=w1_sb[:], in_=w1T[:, D0 : D0 + size, :])
            o_sb = op_pool.tile([128, NQ, B], f16, name=f"o_{size}")
            sg_state[sgi] = (x_sb, w1_sb, o_sb)
        x_sb, w1_sb, _ = sg_state[sgi]
        if act_quad:
            if pr == 0:
                pq = ps1.tile([H, QUAD * B], f32, name="p_quad")
                quad_p[(sgi, c)] = pq
            p1 = quad_p[(sgi, c)][:, pr * PAIR * B : (pr + 1) * PAIR * B]
        else:
            p1 = ps1.tile([H, PAIR * B], f32)
        for j in range(PAIR):
            jj = PAIR * pr + j
            nc.tensor.matmul(
                p1[:, j * B : (j + 1) * B],
                lhsT=w1_sb[:, NQ * jj + c, :],
                rhs=x_sb[:, NQ * jj + c, :],
                start=True,
                stop=True,
            )
        return p1

    def emit_consume(pi, p1):
        """Stage 2: gelu; at quad end also MM2 -> DVE copy -> out DMA."""
        sgi, c, pr = pairs[pi]
        D0, size = sgs[sgi]
        NQ = size // QUAD
        _, _, o_sb = sg_state[sgi]
        if pr == 0:
            g_new = gp.tile([H, QUAD * B], f16, name="g_quad")
            quad_g[(sgi, c)] = g_new
        g_sb = quad_g[(sgi, c)]
        if act_quad and pr != NPAIR_Q - 1:
            return  # whole-quad gelu happens on the last pair
        if bias_mode:
            prs = range(NPAIR_Q) if act_quad else [pr]
            pq = quad_p[(sgi, c)] if act_quad else None
            for pr2 in prs:
                psl = (pq[:, pr2 * PAIR * B : (pr2 + 1) * PAIR * B]
                       if act_quad else p1)
                for j in range(PAIR):
                    dd = D0 + NQ * (PAIR * pr2 + j) + c
                    nc.scalar.activation(
                        g_sb[:, (PAIR * pr2 + j) * B : (PAIR * pr2 + j + 1) * B],
                        psl[:, j * B : (j + 1) * B],
                        GELU,
                        bias=b1_sb[:, dd : dd + 1],
                    )
        elif act_quad:
            pq = quad_p[(sgi, c)]
            nc.scalar.activation(g_sb[:], pq[:], GELU)
        else:
            gsl = g_sb[:, pr * PAIR * B : (pr + 1) * PAIR * B]
            nc.scalar.activation(gsl[:], p1[:], GELU)
        if pr != NPAIR_Q - 1:
            return
        # quad complete: MM2 (4 col-tiled 1-row matmuls) + DVE copy
        del quad_g[(sgi, c)]
        if act_quad:
            # reuse the (consumed) last bank of the quad psum tile
            pq = quad_p.pop((sgi, c))
            p2 = pq[:, (QUAD - 1) * B : QUAD * B]
        else:
            p2 = ps2.tile([128, B], f32)
        for j in range(QUAD):
            dd = D0 + NQ * j + c
            nc.tensor.matmul(
                p2[32 * j : 32 * j + 1, :],
                lhsT=w2_sb[:, dd : dd + 1],
                rhs=g_sb[:, j * B : (j + 1) * B],
                start=True,
                stop=True,
                tile_position=(0, 32 * j),
            )
        nc.vector.tensor_copy(o_sb[:, c, :], p2[:])
        if c == NQ - 1:
            nc.sync.dma_start(
                out=outT[D0 : D0 + size, :], in_=o_sb[0::32, :, :]
            )
            del sg_state[sgi]

    # 1-deep software pipeline at pair granularity: PE runs MM1(p+1)
    # while ACT consumes pair p.
    prev = emit_mm1(0)
    for pi in range(len(pairs)):
        if pi + 1 < len(pairs):
            nxt = emit_mm1(pi + 1)
        emit_consume(pi, prev)
        prev = nxt if pi + 1 < len(pairs) else None


def prepare_in_maps(x, W1, b1, W2, x_int8: bool = X_INT8):
    """Host-side shard + transpose (+ int8 quantization). 8 per-core dicts."""
    x = np.asarray(x, dtype=np.float32)
    W1 = np.asarray(W1, dtype=np.float32)
    b1 = np.asarray(b1, dtype=np.float32)
    W2 = np.asarray(W2, dtype=np.float32)

    in_maps = []
    for k in range(N_CORES):
        sl = slice(k * D_LOC, (k + 1) * D_LOC)
        xk = x[:, sl, :]  # [B, D_LOC, M]
        w1k = W1[sl]      # [D_LOC, H, M]
        if x_int8:
            sx = np.abs(xk).max(axis=0) / 127.0          # [D_LOC, M]
            sx = np.maximum(sx, 1e-12)
            xq = np.rint(xk / sx[None]).astype(np.int8)  # [B, D_LOC, M]
            xT_k = np.ascontiguousarray(xq.transpose(2, 1, 0))          # [M, D_LOC, B]
            w1s = w1k * sx[:, None, :]                   # [D_LOC, H, M] * sx[d,m]
        else:
            xT_k = np.ascontiguousarray(
                xk.transpose(2, 1, 0), dtype=np.float16
            )
            w1s = w1k
        w1T_k = np.ascontiguousarray(w1s.transpose(2, 0, 1), dtype=np.float16)  # [M, D_LOC, H]
        w2T_k = np.ascontiguousarray(W2[sl].T, dtype=np.float16)
        b1T_k = np.ascontiguousarray(b1[sl].T, dtype=np.float32)
        in_maps.append({"xT": xT_k, "w1T": w1T_k, "w2T": w2T_k, "b1T": b1T_k})
    return in_maps


def assemble_output(results, b2):
    outT_full = np.concatenate([r["outT"] for r in results], axis=0)  # [D, B] f16
    out = outT_full.T.astype(np.float32)  # [B, D]
    b2 = np.asarray(b2, dtype=np.float32)
    if np.any(b2):
        out = out + b2[None, :]
    return np.ascontiguousarray(out)


def kernel(pre_activation_history, W1, b1, W2, b2):
    from concourse.bass_utils import run_bass_kernel_spmd

    b1 = np.asarray(b1, dtype=np.float32)
    bias_mode = bool(np.any(b1))
    nc = build_nc(bias_mode)
    in_maps = prepare_in_maps(pre_activation_history, W1, b1, W2)
    res = run_bass_kernel_spmd(nc, in_maps, core_ids=list(range(N_CORES)))
    return assemble_output(res.results, b2)


# revision 24
# speedup vs baseline: 1.4324x; 1.0545x over previous
"""Per-neuron grouped MLP (conv-style) kernel for Trainium2, 8 NeuronCores.

Math (per group d):  h = x[:, d, :] @ W1[d].T + b1[d]; g = gelu(h); out[:, d] = g @ W2[d] + b2[d]
  x: [B=512, D=2048, M=128], W1: [D, H=128, M], b1: [D, H], W2: [D, H], b2: [D]

Strategy (v5) — the kernel is ScalarE-bound: gelu on ACT runs at
1 elem/lane/cycle @1.2GHz regardless of dtype (HW-measured ~1000ns per
[128,1024] activation, (172+FD)/1.2), so B*D_LOC*H/128 = 131072
elems/partition set a ~110us/core streaming floor; the schedule
minimizes ACT instruction count (overhead ~143ns/inst) and hides every
other engine under the gelu stream.

  - Shard on D: each of 8 cores owns D_LOC = 256 independent per-neuron MLPs.
  - x is quantized to int8 on host with a per-(d,m) scale folded into W1
    (W1'[m,d,h] = W1[d,h,m] * sx[d,m]); nc.gpsimd.dma_start casts
    int8 -> fp16 in the SDMA datapath, so HBM traffic for x is 1 byte/elem
    while the matmul runs in fp16 on integer-valued activations
    (rel err ~7e-3, gate 2e-2).
  - DRAM layouts are [M, D_LOC, *] so every per-supergroup DMA reads a
    contiguous per-partition chunk.
  - Supergroups of 12 d's (divisible by 3 for ACT triples and by 4 for
    MM2 quads; one leading 4-sg fills the pipeline fast). Per triple of
    3 d's: 3 MM1 matmuls into one [H, 3B] fp32 psum tile (3 banks,
    bufs=2), ONE Gelu (exact erf) psum -> fp16 g_sg in SBUF.
  - After the sg's last gelu: per quad c (d = D0+3j+c), 4 packed MM2s
    with tile_position=(0,32j) run concurrently in PE column groups ->
    psum rows {0,32,64,96}; DVE copies psum -> fp16 o_sb; one
    strided-partition DMA writes outT[D0:D0+12] (row order (j,c) == d).
    b2 and the fp32 upcast happen on host.
  - PSUM budget: p_tri 2x3 + p_one 1x1 + ps2 1x1 = 8 banks exactly.
  - Software pipelining at triple granularity (MM1 of unit u+1 emitted
    before the gelu of unit u) keeps ACT streaming gaplessly; a dummy
    1-col gelu hoists the ~2.7us ACT_TABLE_LOAD under the initial DMAs.
  - Variants measured and rejected: quad-level ACT in 4-bank tiles
    (Tile's whole-tile dependency tracking serializes the next MM1
    behind the DVE read of the shared tile: 165us vs 125us); fp8
    anywhere (3.6% rms error fails the gate); gelu on DVE/GPSIMD
    (no LUT; polynomial needs ~10 passes at worse throughput).
"""

import numpy as np

B, D, M, H = 512, 2048, 128, 128
N_CORES = 8
D_LOC = D // N_CORES  # 256
QUAD = 4     # d's per MM2 packing group
TRI = 3      # d's per psum1/ACT batch in 12-sgs
SUPER = 12   # d's per super-group: one x DMA, one w1 DMA, one out DMA

X_INT8 = True  # False: ship x as fp16 (no quantization)

_NC_CACHE = {}


def build_nc(bias_mode: bool, x_int8: bool = X_INT8, reps: int = 1):
    key = (bias_mode, x_int8, reps)
    if key in _NC_CACHE:
        return _NC_CACHE[key]

    import concourse.bacc as bacc
    import concourse.mybir as mybir
    import concourse.tile as tile

    f32 = mybir.dt.float32
    f16 = mybir.dt.float16
    xdt = mybir.dt.int8 if x_int8 else f16
    GELU = mybir.ActivationFunctionType.Gelu

    nc = bacc.Bacc("TRN2", target_bir_lowering=False, debug=False, num_devices=N_CORES)
    xT = nc.dram_tensor("xT", [M, D_LOC, B], xdt, kind="ExternalInput").ap()
    w1T = nc.dram_tensor("w1T", [M, D_LOC, H], f16, kind="ExternalInput").ap()
    w2T = nc.dram_tensor("w2T", [H, D_LOC], f16, kind="ExternalInput").ap()
    b1T = nc.dram_tensor("b1T", [H, D_LOC], f32, kind="ExternalInput").ap()
    outT = nc.dram_tensor("outT", [D_LOC, B], f16, kind="ExternalOutput").ap()

    with (
        tile.TileContext(nc) as tc,
        tc.tile_pool(name="singles", bufs=1) as singles,
        tc.tile_pool(name="xp", bufs=3) as xp,
        tc.tile_pool(name="wp", bufs=2) as wp,
        tc.tile_pool(name="gp", bufs=3) as gp,
        tc.tile_pool(name="op", bufs=4) as op_pool,
        tc.tile_pool(name="ps1", bufs=2, space="PSUM") as ps1,
        tc.tile_pool(name="ps2", bufs=1, space="PSUM") as ps2,
    ):
        w2_sb = singles.tile([H, D_LOC], f16)
        nc.sync.dma_start(out=w2_sb[:], in_=w2T[:])
        b1_sb = None
        if bias_mode:
            b1_sb = singles.tile([H, D_LOC], f32)
            nc.sync.dma_start(out=b1_sb[:], in_=b1T[:])
        # Dummy 1-col gelu with no input deps: hoists the ~2.7us
        # ACT_TABLE_LOAD for the gelu set under the initial DMAs.
        warm = singles.tile([128, 1], f32)
        nc.gpsimd.memset(warm[:], 0.0)
        nc.scalar.activation(warm[:], warm[:], GELU)

        for _rep in range(reps):
            _body_loop(nc, tc, bias_mode, f16, f32, GELU,
                       xT, w1T, outT, w2_sb, b1_sb,
                       xp, wp, gp, op_pool, ps1, ps2)

    nc.compile()
    _NC_CACHE[key] = nc
    return nc


def _sg_plan():
    """(D0, size) list: one leading 4-sg (single-d ACTs, fast pipeline
    fill after a 256KB DMA), then 21 supergroups of 12 (triple ACTs)."""
    sizes = [4] + [SUPER] * ((D_LOC - 4) // SUPER)
    assert sum(sizes) == D_LOC
    out, d0 = [], 0
    for s in sizes:
        out.append((d0, s))
        d0 += s
    return out


def _body_loop(nc, tc, bias_mode, f16, f32, GELU, xT, w1T, outT, w2_sb, b1_sb,
               xp, wp, gp, op_pool, ps1, ps2):
    sgs = _sg_plan()
    # unit list: (sgi, u, n_d) — n_d MM1s + one gelu per unit
    units = []
    for sgi, (D0, size) in enumerate(sgs):
        if size % TRI == 0:
            units += [(sgi, u, TRI) for u in range(size // TRI)]
        else:
            units += [(sgi, u, 1) for u in range(size)]

    sg_state = {}   # sgi -> (x_sb, w1_sb, o_sb, g_sb)

    def emit_mm1(ui):
        """Stage 1: (DMA loads at supergroup start) + n_d MM1 matmuls."""
        sgi, u, n_d = units[ui]
        D0, size = sgs[sgi]
        if u == 0:
            x_sb = xp.tile([M, size, B], f16, name=f"x_{size}")
            nc.gpsimd.dma_start(out=x_sb[:], in_=xT[:, D0 : D0 + size, :])
            w1_sb = wp.tile([M, size, H], f16, name=f"w1_{size}")
            nc.sync.dma_start(out=w1_sb[:], in_=w1T[:, D0 : D0 + size, :])
            o_sb = op_pool.tile([128, size // QUAD, B], f16, name=f"o_{size}")
            g_sb = gp.tile([H, size, B], f16, name=f"g_{size}")
            sg_state[sgi] = (x_sb, w1_sb, o_sb, g_sb)
        x_sb, w1_sb, _, _ = sg_state[sgi]
        p1 = ps1.tile([H, n_d * B], f32, name=f"p_{n_d}",
                      bufs=2 if n_d == TRI else 1)
        for k in range(n_d):
            dl = n_d * u + k  # local d index within the supergroup
            nc.tensor.matmul(
                p1[:, k * B : (k + 1) * B],
                lhsT=w1_sb[:, dl, :],
                rhs=x_sb[:, dl, :],
                start=True,
                stop=True,
            )
        return p1

    def emit_consume(ui, p1):
        """Stage 2: gelu; after the sg's last unit: MM2 quads -> DVE -> DMA."""
        sgi, u, n_d = units[ui]
        D0, size = sgs[sgi]
        NQ = size // QUAD
        _, _, o_sb, g_sb = sg_state[sgi]
        gsl = g_sb[:, n_d * u : n_d * (u + 1), :]
        if bias_mode:
            for k in range(n_d):
                dd = D0 + n_d * u + k
                nc.scalar.activation(
                    gsl[:, k, :],
                    p1[:, k * B : (k + 1) * B],
                    GELU,
                    bias=b1_sb[:, dd : dd + 1],
                )
        else:
            nc.scalar.activation(gsl[:], p1[:], GELU)
        if n_d * (u + 1) != size:
            return
        # supergroup's gelus complete: MM2 quads (col-tiled, concurrent),
        # DVE copies, one strided out DMA. Quad c: d = D0 + NQ*j + c.
        del sg_state[sgi]
        for c in range(NQ):
            p2 = ps2.tile([128, B], f32)
            for j in range(QUAD):
                dl = NQ * j + c
                nc.tensor.matmul(
                    p2[32 * j : 32 * j + 1, :],
                    lhsT=w2_sb[:, D0 + dl : D0 + dl + 1],
                    rhs=g_sb[:, dl, :],
                    start=True,
                    stop=True,
                    tile_position=(0, 32 * j),
                )
            nc.vector.tensor_copy(o_sb[:, c, :], p2[:])
        nc.sync.dma_start(
            out=outT[D0 : D0 + size, :], in_=o_sb[0::32, :, :]
        )

    # 1-deep software pipeline at unit granularity: PE runs MM1(u+1)
    # while ACT consumes unit u.
    prev = emit_mm1(0)
    for ui in range(len(units)):
        if ui + 1 < len(units):
            nxt = emit_mm1(ui + 1)
        emit_consume(ui, prev)
        prev = nxt if ui + 1 < len(units) else None


def prepare_in_maps(x, W1, b1, W2, x_int8: bool = X_INT8):
    """Host-side shard + transpose (+ int8 quantization). 8 per-core dicts."""
    x = np.asarray(x, dtype=np.float32)
    W1 = np.asarray(W1, dtype=np.float32)
    b1 = np.asarray(b1, dtype=np.float32)
    W2 = np.asarray(W2, dtype=np.float32)

    in_maps = []
    for k in range(N_CORES):
        sl = slice(k * D_LOC, (k + 1) * D_LOC)
        xk = x[:, sl, :]  # [B, D_LOC, M]
        w1k = W1[sl]      # [D_LOC, H, M]
        if x_int8:
            sx = np.abs(xk).max(axis=0) / 127.0          # [D_LOC, M]
            sx = np.maximum(sx, 1e-12)
            xq = np.rint(xk / sx[None]).astype(np.int8)  # [B, D_LOC, M]
            xT_k = np.ascontiguousarray(xq.transpose(2, 1, 0))          # [M, D_LOC, B]
            w1s = w1k * sx[:, None, :]                   # [D_LOC, H, M] * sx[d,m]
        else:
            xT_k = np.ascontiguousarray(
                xk.transpose(2, 1, 0), dtype=np.float16
            )
            w1s = w1k
        w1T_k = np.ascontiguousarray(w1s.transpose(2, 0, 1), dtype=np.float16)  # [M, D_LOC, H]
        w2T_k = np.ascontiguousarray(W2[sl].T, dtype=np.float16)
        b1T_k = np.ascontiguousarray(b1[sl].T, dtype=np.float32)
        in_maps.append({"xT": xT_k, "w1T": w1T_k, "w2T": w2T_k, "b1T": b1T_k})
    return in_maps


def assemble_output(results, b2):
    outT_full = np.concatenate([r["outT"] for r in results], axis=0)  # [D, B] f16
    out = outT_full.T.astype(np.float32)  # [B, D]
    b2 = np.asarray(b2, dtype=np.float32)
    if np.any(b2):
        out = out + b2[None, :]
    return np.ascontiguousarray(out)


def kernel(pre_activation_history, W1, b1, W2, b2):
    from concourse.bass_utils import run_bass_kernel_spmd

    b1 = np.asarray(b1, dtype=np.float32)
    bias_mode = bool(np.any(b1))
    nc = build_nc(bias_mode)
    in_maps = prepare_in_maps(pre_activation_history, W1, b1, W2)
    res = run_bass_kernel_spmd(nc, in_maps, core_ids=list(range(N_CORES)))
    return assemble_output(res.results, b2)
